# revision 1
# baseline (speedup 1.0000x reference)
"""Trainium2 Bass kernel for nn_Attention (dense transformer block):
y = Attention(RoPE(x@wqT), RoPE(x@wkT), x@wvT, causal) @ woT

Sharding: 8 cores = 2 batches x 4 head-groups (tensor-parallel heads,
data-parallel batch).  Each core handles one batch and 4 of the 16 heads
(512 of the 2048 channels): column-shard of wq/wk/wv, row-shard of wo.
Each core emits a full-shape [S, D] partial of y; the host sums the 4
partials per batch.

Kernel layout strategy (per core, SPMD — identical program, per-core data):
  - qT/kT computed directly in [head_dim, seq] layout (lhsT = wqT slice,
    rhs = xT streamed from DRAM).  RoPE pair-swap done with a DVE
    stream_shuffle + DVE/GpSimd combine against sign-folded cos/sin tables.
  - v computed in natural [seq, head_dim] layout (lhsT = xV column block,
    a host-retiled copy of x that makes those loads contiguous).
  - scores computed transposed: sT[sk, sq] = kT_tile.T @ qT_chunk, so the
    PV matmul needs no transposes.  Softmax runs without max subtraction
    (scores are bounded, |s*scale| < ~6); denominators by summing the prob
    tiles elementwise on the otherwise-idle GpSimd engine (final add on DVE
    to produce the fp32r tag), then a single all-ones [128,128] stationary
    matmul per chunk reduces over partitions and leaves the sum broadcast.
  - causal masking: off-diagonal upper tiles skipped entirely; the 4
    diagonal-straddling tile shapes multiply post-exp by host-built 0/1
    masks.
  - all matmuls run as float32r (full-rate fp32 path on the PE).
"""

import os
import sys

import numpy as np

for _p in ("/opt/trn_rl_repo", "/root/.axon_site/_ro/trn_rl_repo"):
    if os.path.isdir(_p) and _p not in sys.path:
        sys.path.insert(0, _p)

import concourse.bass as bass
import concourse.tile as tile
from concourse import bacc
from concourse import mybir
from concourse import bass_utils

B, S, D, H = 2, 2048, 2048, 16
HD = 128                 # head dim
HPC = 4                  # heads per core
CPB = 4                  # cores per batch
N_CORES = 8
NK = D // 128            # 16 contraction chunks
NSQ = S // 512           # 4 sq chunks of 512
NSK = S // 128           # 16 sk tiles of 128
SCALE = float(1.0 / np.sqrt(np.float32(HD)))

F32 = mybir.dt.float32
F32R = mybir.dt.float32r
USE_F32R = True

EXP = mybir.ActivationFunctionType.Exp
SWAP_MASK = [i ^ 1 for i in range(32)]


MMDT = F32R if USE_F32R else F32


def round_fp32r(x):
    """Round fp32 array to fp32r (e8m11) with round-to-nearest-even."""
    if not USE_F32R:
        return np.ascontiguousarray(x, dtype=np.float32)
    v = np.ascontiguousarray(x, np.float32).view(np.uint32)
    b = (v >> 12) & 1
    v = (v + 0x7FF + b) & np.uint32(0xFFFFF000)
    return v.view(np.float32)


def _emit(tc):
    nc = tc.nc

    xT = nc.dram_tensor("xT", [D, S], MMDT, kind="ExternalInput").ap()
    xV = nc.dram_tensor("xV", [S, D], MMDT, kind="ExternalInput").ap()
    wqT = nc.dram_tensor("wqT", [D, HPC * HD], MMDT, kind="ExternalInput").ap()
    wkT = nc.dram_tensor("wkT", [D, HPC * HD], MMDT, kind="ExternalInput").ap()
    wvT = nc.dram_tensor("wvT", [D, HPC * HD], MMDT, kind="ExternalInput").ap()
    woT = nc.dram_tensor("woT", [HPC * HD, D], MMDT, kind="ExternalInput").ap()
    cosq = nc.dram_tensor("cosq", [HD, S], F32, kind="ExternalInput").ap()
    sinq = nc.dram_tensor("sinq", [HD, S], F32, kind="ExternalInput").ap()
    dmask = nc.dram_tensor("dmask", [4, 128, 512], MMDT, kind="ExternalInput").ap()
    onesd = nc.dram_tensor("onesd", [128, 128], MMDT, kind="ExternalInput").ap()
    y = nc.dram_tensor("y", [S, D], F32, kind="ExternalOutput").ap()

    # two DMA issue queues: SP for the latency-critical stream, ACT for the rest
    dma_a = nc.sync
    dma_b = nc.scalar

    # long-lived pools first (stack allocator wants LIFO release order)
    consts = tc.alloc_tile_pool(name="consts", bufs=1)
    qk_pool = tc.alloc_tile_pool(name="qkp", bufs=HPC)
    qT = [qk_pool.tile([128, S], MMDT, name=f"qT{h}", tag="qT") for h in range(HPC)]
    kT = [qk_pool.tile([128, S], MMDT, name=f"kT{h}", tag="kT") for h in range(HPC)]

    # ---- phase 1a (merged): q and k projections (+RoPE), single x pass
    ones_sq = consts.tile([128, 128], MMDT, name="ones_sq")
    mask_sb = []
    for m in range(4):
        mt = consts.tile([128, 512], MMDT, name=f"mask{m}", tag=f"mask{m}")
        mask_sb.append(mt)
    ropec = tc.alloc_tile_pool(name="ropec", bufs=1)
    cos_sb = ropec.tile([128, S], F32, name="cos_sb")
    sin_sb = ropec.tile([128, S], F32, name="sin_sb")
    tpool = tc.alloc_tile_pool(name="tqk", bufs=2)

    wk_pool = tc.alloc_tile_pool(name="wkp", bufs=NK)
    xpool = tc.alloc_tile_pool(name="xqk", bufs=NK + 2)
    wq_pool = tc.alloc_tile_pool(name="wqp", bufs=NK)
    pspool = tc.alloc_tile_pool(name="psqk", bufs=8, space="PSUM")

    # interleave weight and first-chunk x loads so the k-loop starts early
    wq_sb, wk_sb, xs0 = [], [], []
    for k in range(NK):
        wt = wq_pool.tile([128, HPC * HD], MMDT, name=f"wq{k}", tag="wq")
        dma_a.dma_start(out=wt, in_=wqT[128 * k:128 * (k + 1), :])
        wq_sb.append(wt)
        xt = xpool.tile([128, 512], MMDT, name=f"x_0_{k}", tag="xs")
        eng = dma_b if k % 2 == 0 else dma_a
        eng.dma_start(out=xt, in_=xT[128 * k:128 * (k + 1), 0:512])
        xs0.append(xt)
        wt = wk_pool.tile([128, HPC * HD], MMDT, name=f"wk{k}", tag="wk")
        dma_b.dma_start(out=wt, in_=wkT[128 * k:128 * (k + 1), :])
        wk_sb.append(wt)
    # rope/mask constants arrive behind the first chunk's stream
    dma_b.dma_start(out=cos_sb, in_=cosq)
    dma_b.dma_start(out=sin_sb, in_=sinq)
    dma_b.dma_start(out=ones_sq, in_=onesd)
    for m in range(4):
        dma_b.dma_start(out=mask_sb[m], in_=dmask[m])


    for j in range(NSQ):
        sl = slice(512 * j, 512 * (j + 1))
        if j == 0:
            xs = xs0
        else:
            xs = []
            for k in range(NK):
                xt = xpool.tile([128, 512], MMDT, name=f"x_{j}_{k}", tag="xs")
                eng = dma_a if k % 2 == 0 else dma_b
                eng.dma_start(out=xt, in_=xT[128 * k:128 * (k + 1), sl])
                xs.append(xt)
        for w_sb, dsts, tagn in ((wq_sb, qT, "q"), (wk_sb, kT, "k")):
            accs = [
                pspool.tile(
                    [128, 512], F32, name=f"acc{tagn}_{j}_{h}", tag="acc"
                )
                for h in range(HPC)
            ]
            for k in range(NK):
                for h in range(HPC):
                    nc.tensor.matmul(
                        accs[h], w_sb[k][:, 128 * h:128 * (h + 1)], xs[k],
                        start=(k == 0), stop=(k == NK - 1),
                    )
            for h in range(HPC):
                acc, dst = accs[h], dsts[h]
                raw = tpool.tile([128, 512], F32, name=f"raw{tagn}_{j}_{h}", tag="raw")
                nc.vector.tensor_copy(out=raw, in_=acc)
                shuf = tpool.tile([128, 512], F32, name=f"sh{tagn}_{j}_{h}", tag="shuf")
                nc.vector.stream_shuffle(shuf, acc, SWAP_MASK)
                t1 = tpool.tile([128, 512], F32, name=f"t1{tagn}_{j}_{h}", tag="t1")
                nc.vector.tensor_mul(t1, shuf, sin_sb[:, sl])
                t2 = tpool.tile([128, 512], F32, name=f"t2{tagn}_{j}_{h}", tag="t2")
                nc.gpsimd.tensor_mul(t2, raw, cos_sb[:, sl])
                nc.vector.tensor_add(dst[:, sl], t1, t2)
    wq_pool.release()

    # ---- phase 1b: v projection in natural [seq, head_dim] layout
    # wv tiles recycle the wk pool's slots; accv tiles recycle the ph1a psum
    # tag — both avoid pool-boundary serialization at the phase seam.
    wv_sb = []
    for k in range(NK):
        wt = wk_pool.tile([128, HPC * HD], MMDT, name=f"wv{k}", tag="wk")
        dma_b.dma_start(out=wt, in_=wvT[128 * k:128 * (k + 1), :])
        wv_sb.append(wt)

    v_pool = tc.alloc_tile_pool(name="vp", bufs=NSK, side="right")
    v_sb = [v_pool.tile([128, HPC * HD], MMDT, name=f"v{m}", tag="v") for m in range(NSK)]
    for m in range(NSK):
        xcp = []
        for g in range(4):
            xt = xpool.tile([128, 4, 128], MMDT, name=f"xc{m}_{g}", tag="xs")
            eng = dma_a if g % 2 == 0 else dma_b
            eng.dma_start(
                out=xt,
                in_=xV[128 * m:128 * (m + 1), 512 * g:512 * (g + 1)].rearrange(
                    "p (kt c) -> p kt c", c=128
                ),
            )
            xcp.append(xt)
        acc = pspool.tile([128, HPC * HD], F32, name=f"accv{m}", tag="acc")
        for k in range(NK):
            nc.tensor.matmul(
                acc, xcp[k // 4][:, k % 4, :], wv_sb[k],
                start=(k == 0), stop=(k == NK - 1),
            )
        nc.vector.tensor_copy(out=v_sb[m], in_=acc)
    xpool.release()
    pspool.release()
    wk_pool.release()
    tpool.release()
    ropec.release()

    # ---- phase 2: causal attention per head, transposed-score layout
    oh_pool = tc.alloc_tile_pool(name="ohp", bufs=HPC, side="right")
    out_hT = [oh_pool.tile([128, S], MMDT, name=f"oh{h}", tag="oh") for h in range(HPC)]
    # prefetch wo during attention
    wo_pool = tc.alloc_tile_pool(name="wop", bufs=HPC, side="right")
    wo_sb = []
    for h in range(HPC):
        wt = wo_pool.tile([128, D], MMDT, name=f"wo{h}", tag="wo")
        dma_a.dma_start(out=wt, in_=woT[128 * h:128 * (h + 1), :])
        wo_sb.append(wt)

    pp = tc.alloc_tile_pool(name="pp", bufs=8)
    small2 = tc.alloc_tile_pool(name="small2", bufs=4)
    pss = tc.alloc_tile_pool(name="pss", bufs=5, space="PSUM")
    psd = tc.alloc_tile_pool(name="psd", bufs=1, space="PSUM")
    pspv = tc.alloc_tile_pool(name="pspv", bufs=2, space="PSUM")
    for h in range(HPC):
        for j in range(NSQ):
            sl = slice(512 * j, 512 * (j + 1))
            nsk = 4 * j + 4
            den = psd.tile([128, 512], F32, name=f"den{h}_{j}", tag="den")
            pv = pspv.tile([128, 512], F32, name=f"pv{h}_{j}", tag="pv")
            # diagonal (masked) tiles first: their exp->mask latency hides
            # under the unmasked tiles' pv matmuls.  The denominator is the
            # elementwise sum of all pt tiles (GpSimd running adds, last add
            # on DVE to produce the fp32r tag) reduced by ONE ones-matmul.
            order = list(range(4 * j, nsk)) + list(range(0, 4 * j))
            # diagonal tile with mask pattern m: columns sql < 128*m are
            # fully masked — compute only a column slice (kept >= 256 wide
            # so fp32r stays at full rate; m=3 pays 128 wasted columns)
            offs = {0: 0, 1: 128, 2: 256, 3: 256}
            pacc = None
            pts = []
            for idx, i in enumerate(order):
                off = offs[i - 4 * j] if i >= 4 * j else 0
                cs = slice(off, 512)
                qs = slice(512 * j + off, 512 * (j + 1))
                s_ps = pss.tile([128, 512], F32, name=f"s{h}_{j}_{i}", tag="s")
                nc.tensor.matmul(
                    s_ps[:, cs], kT[h][:, 128 * i:128 * (i + 1)], qT[h][:, qs],
                    start=True, stop=True,
                )
                pt = pp.tile([128, 512], MMDT, name=f"p{h}_{j}_{i}", tag="pt")
                nc.scalar.activation(pt[:, cs], s_ps[:, cs], EXP, bias=0.0, scale=SCALE)
                if i >= 4 * j:
                    nc.vector.tensor_mul(pt[:, cs], pt[:, cs], mask_sb[i - 4 * j][:, cs])
                nc.tensor.matmul(
                    pv[:, cs], v_sb[i][:, 128 * h:128 * (h + 1)], pt[:, cs],
                    start=(idx == 0), stop=(idx == nsk - 1),
                )
                pts.append((pt, off))
                if idx == 1:
                    pacc = small2.tile(
                        [128, 512], F32, name=f"pa{h}_{j}", tag="pacc", bufs=2
                    )
                    nc.gpsimd.tensor_copy(out=pacc, in_=pts[0][0])
                    o1 = pts[1][1]
                    nc.gpsimd.tensor_add(
                        pacc[:, o1:], pacc[:, o1:], pt[:, o1:]
                    )
                elif 1 < idx < nsk - 1:
                    nc.gpsimd.tensor_add(pacc[:, off:], pacc[:, off:], pt[:, cs])
                elif idx == nsk - 1:
                    pacc_r = small2.tile(
                        [128, 512], MMDT, name=f"par{h}_{j}", tag="paccr", bufs=2
                    )
                    if off > 0:
                        nc.vector.tensor_copy(out=pacc_r[:, 0:off], in_=pacc[:, 0:off])
                    nc.vector.tensor_add(
                        pacc_r[:, cs], pacc[:, cs], pt[:, cs]
                    )
                    nc.tensor.matmul(den, ones_sq, pacc_r, start=True, stop=True)
            recip = small2.tile([128, 512], F32, name=f"rc{h}_{j}", tag="recip")
            nc.vector.reciprocal(recip, den)
            nc.vector.tensor_mul(out_hT[h][:, sl], pv, recip)
    pspv.release()
    psd.release()
    pss.release()
    small2.release()
    pp.release()
    qk_pool.release()

    # ---- phase 3: row-parallel wo partial product, row-block output DMAs
    ys_pool = tc.alloc_tile_pool(name="ysp", bufs=3)
    psy_pool = tc.alloc_tile_pool(name="psy", bufs=3, space="PSUM")
    for t in range(NSK):
        ys = ys_pool.tile([128, D], F32, name=f"ys{t}", tag="ys")
        for n in range(NSQ):
            acc = psy_pool.tile([128, 512], F32, name=f"accy{t}_{n}", tag="y")
            for h in range(HPC):
                nc.tensor.matmul(
                    acc,
                    out_hT[h][:, 128 * t:128 * (t + 1)],
                    wo_sb[h][:, 512 * n:512 * (n + 1)],
                    start=(h == 0),
                    stop=(h == HPC - 1),
                )
            nc.vector.tensor_copy(out=ys[:, 512 * n:512 * (n + 1)], in_=acc)
        dma_a.dma_start(out=y[128 * t:128 * (t + 1), :], in_=ys)
    psy_pool.release()
    ys_pool.release()
    wo_pool.release()
    oh_pool.release()
    v_pool.release()
    consts.release()


_PROGRAM = None


def build_program():
    global _PROGRAM
    if _PROGRAM is None:
        nc = bacc.Bacc("TRN2", target_bir_lowering=False, debug=False)
        with tile.TileContext(nc) as tc:
            _emit(tc)
        nc.compile()
        _PROGRAM = nc
    return _PROGRAM


def make_core_inputs(x, freqs_cos, freqs_sin, wq, wk, wv, wo):
    """Host-side sharding: returns list of 8 per-core input dicts."""
    x = np.asarray(x, dtype=np.float32)
    freqs_cos = np.asarray(freqs_cos, dtype=np.float32)
    freqs_sin = np.asarray(freqs_sin, dtype=np.float32)
    wq = np.asarray(wq, dtype=np.float32)
    wk = np.asarray(wk, dtype=np.float32)
    wv = np.asarray(wv, dtype=np.float32)
    wo = np.asarray(wo, dtype=np.float32)

    cosq = np.ascontiguousarray(np.repeat(freqs_cos.T, 2, axis=0))  # [128, S]
    sinq = np.ascontiguousarray(np.repeat(freqs_sin.T, 2, axis=0))
    sinq[0::2, :] *= -1.0  # even rows: -sin; odd rows: +sin

    skl = np.arange(128)[:, None]
    sql = np.arange(512)[None, :]
    dmask = np.stack(
        [(128 * m + skl <= sql).astype(np.float32) for m in range(4)]
    )  # [4, 128, 512]

    onesd = np.ones((128, 128), dtype=np.float32)
    xTs = [round_fp32r(x[b].T) for b in range(B)]
    # V-phase layout: xV[128m+p, 128kt+c] = x[b][128m+c, 128kt+p]
    xVs = [
        np.ascontiguousarray(
            xr.T.reshape(16, 128, 16, 128).transpose(0, 3, 2, 1).reshape(2048, 2048)
        )
        for xr in xTs
    ]
    in_maps = []
    for c in range(N_CORES):
        b, g = divmod(c, CPB)
        hsl = slice(512 * g, 512 * (g + 1))
        in_maps.append(
            {
                "xT": xTs[b],
                "xV": xVs[b],
                "wqT": round_fp32r(wq[hsl, :].T),
                "wkT": round_fp32r(wk[hsl, :].T),
                "wvT": round_fp32r(wv[hsl, :].T),
                "woT": round_fp32r(wo[:, hsl].T),
                "cosq": cosq,
                "sinq": sinq,
                "dmask": dmask,
                "onesd": onesd,
            }
        )
    return in_maps


def run(inputs, trace=False, **spmd_kwargs):
    """Run the SPMD kernel on 8 cores.  Returns (y_full, BassKernelResults)."""
    nc = build_program()
    in_maps = make_core_inputs(
        inputs["x"], inputs["freqs_cos"], inputs["freqs_sin"],
        inputs["wq"], inputs["wk"], inputs["wv"], inputs["wo"],
    )
    res = bass_utils.run_bass_kernel_spmd(
        nc, in_maps, list(range(N_CORES)), trace=trace, **spmd_kwargs
    )
    out = np.zeros((B, S, D), dtype=np.float32)
    for c in range(N_CORES):
        out[c // CPB] += res.results[c]["y"]
    return out, res


def kernel(**inputs):
    out, _ = run(inputs, trace=False)
    return out


def simulate_core(core_idx, inputs):
    """CoreSim-validate a single core's program; returns its partial y."""
    from concourse.bass_interp import CoreSim

    nc = build_program()
    in_maps = make_core_inputs(
        inputs["x"], inputs["freqs_cos"], inputs["freqs_sin"],
        inputs["wq"], inputs["wk"], inputs["wv"], inputs["wo"],
    )
    sim = CoreSim(nc)
    for name, arr in in_maps[core_idx].items():
        sim.tensor(name)[:] = arr
    sim.simulate()
    return np.array(sim.tensor("y"))



# revision 4
# speedup vs baseline: 2.7918x; 2.7918x over previous
"""Trainium2 Bass kernel for nn_Attention (dense transformer block):
y = Attention(RoPE(x@wqT), RoPE(x@wkT), x@wvT, causal) @ woT

Sharding: 8 cores = 2 batches x 4 head-groups (tensor-parallel heads,
data-parallel batch).  Each core handles one batch and 4 of the 16 heads
(512 of the 2048 channels): column-shard of wq/wk/wv, row-shard of wo.
Each core emits a full-shape [S, D] partial of y; the host sums the 4
partials per batch.

v2 (bf16): all matmul operands in bf16 (measured HW absmax-rel ~3e-3,
gate is 2e-2).  PE rate is the same as fp32r (~0.59 ns/row + 42 ns fixed
per instruction, measured), but bf16 halves DMA bytes and doubles
DVE/GpSimd/Act elementwise throughput, pulling those engines off the
attention critical path:
  - RoPE without the PSUM->SBUF staging copy (both consumers read PSUM).
  - exp() writes bf16 prob tiles; denominator accumulated f32 on TWO
    parallel chains (GpSimd evens, DVE odds) instead of one serial
    GpSimd chain (which was ~1us/tile x 144 tiles ~ the attention-phase
    bottleneck), then one ones-matmul per chunk reduces over partitions.
  - reciprocal via the ~5x faster approx-NR path (den is well in range).
"""

import os
import sys

import numpy as np
import ml_dtypes

for _p in ("/opt/trn_rl_repo", "/root/.axon_site/_ro/trn_rl_repo"):
    if os.path.isdir(_p) and _p not in sys.path:
        sys.path.insert(0, _p)

import concourse.bass as bass
import concourse.tile as tile
from concourse import bacc
from concourse import mybir
from concourse import bass_utils

B, S, D, H = 2, 2048, 2048, 16
HD = 128                 # head dim
HPC = 4                  # heads per core
CPB = 4                  # cores per batch
N_CORES = 8
NK = D // 128            # 16 contraction chunks
NSQ = S // 512           # 4 sq chunks of 512
NSK = S // 128           # 16 sk tiles of 128
SCALE = float(1.0 / np.sqrt(np.float32(HD)))

F32 = mybir.dt.float32
BF16 = mybir.dt.bfloat16
NPBF = ml_dtypes.bfloat16

EXP = mybir.ActivationFunctionType.Exp
SWAP_MASK = [i ^ 1 for i in range(32)]


def _emit(tc):
    nc = tc.nc

    xT = nc.dram_tensor("xT", [D, S], BF16, kind="ExternalInput").ap()
    xV = nc.dram_tensor("xV", [S, D], BF16, kind="ExternalInput").ap()
    wqT = nc.dram_tensor("wqT", [D, HPC * HD], BF16, kind="ExternalInput").ap()
    wkT = nc.dram_tensor("wkT", [D, HPC * HD], BF16, kind="ExternalInput").ap()
    wvT = nc.dram_tensor("wvT", [D, HPC * HD], BF16, kind="ExternalInput").ap()
    woT = nc.dram_tensor("woT", [HPC * HD, D], BF16, kind="ExternalInput").ap()
    cosq = nc.dram_tensor("cosq", [HD, S], F32, kind="ExternalInput").ap()
    sinq = nc.dram_tensor("sinq", [HD, S], F32, kind="ExternalInput").ap()
    dmask = nc.dram_tensor("dmask", [4, 128, 512], BF16, kind="ExternalInput").ap()
    onesd = nc.dram_tensor("onesd", [128, 128], BF16, kind="ExternalInput").ap()
    y = nc.dram_tensor("y", [S, D], F32, kind="ExternalOutput").ap()

    # two DMA issue queues: SP for the latency-critical stream, ACT for the rest
    dma_a = nc.sync
    dma_b = nc.scalar

    # long-lived pools first (stack allocator wants LIFO release order)
    consts = tc.alloc_tile_pool(name="consts", bufs=1)
    qk_pool = tc.alloc_tile_pool(name="qkp", bufs=HPC)
    qT = [qk_pool.tile([128, S], BF16, name=f"qT{h}", tag="qT") for h in range(HPC)]
    kT = [qk_pool.tile([128, S], BF16, name=f"kT{h}", tag="kT") for h in range(HPC)]

    # ---- phase 1a (merged): q and k projections (+RoPE), single x pass
    ones_sq = consts.tile([128, 128], BF16, name="ones_sq")
    mask_sb = []
    for m in range(4):
        mt = consts.tile([128, 512], BF16, name=f"mask{m}", tag=f"mask{m}")
        mask_sb.append(mt)
    ropec = tc.alloc_tile_pool(name="ropec", bufs=1)
    cos_sb = ropec.tile([128, S], F32, name="cos_sb")
    sin_sb = ropec.tile([128, S], F32, name="sin_sb")
    tpool = tc.alloc_tile_pool(name="tqk", bufs=3)

    wk_pool = tc.alloc_tile_pool(name="wkp", bufs=NK)
    xpool = tc.alloc_tile_pool(name="xqk", bufs=NK + 2)
    wq_pool = tc.alloc_tile_pool(name="wqp", bufs=NK)
    pspool = tc.alloc_tile_pool(name="psqk", bufs=8, space="PSUM")

    # interleave weight and first-chunk x loads so the k-loop starts early
    wq_sb, wk_sb, xs0 = [], [], []
    for k in range(NK):
        wt = wq_pool.tile([128, HPC * HD], BF16, name=f"wq{k}", tag="wq")
        dma_a.dma_start(out=wt, in_=wqT[128 * k:128 * (k + 1), :])
        wq_sb.append(wt)
        xt = xpool.tile([128, 512], BF16, name=f"x_0_{k}", tag="xs")
        eng = dma_b if k % 2 == 0 else dma_a
        eng.dma_start(out=xt, in_=xT[128 * k:128 * (k + 1), 0:512])
        xs0.append(xt)
        wt = wk_pool.tile([128, HPC * HD], BF16, name=f"wk{k}", tag="wk")
        dma_b.dma_start(out=wt, in_=wkT[128 * k:128 * (k + 1), :])
        wk_sb.append(wt)
    # rope/mask constants arrive behind the first chunk's stream
    dma_b.dma_start(out=cos_sb, in_=cosq)
    dma_b.dma_start(out=sin_sb, in_=sinq)
    dma_b.dma_start(out=ones_sq, in_=onesd)
    for m in range(4):
        dma_b.dma_start(out=mask_sb[m], in_=dmask[m])

    for j in range(NSQ):
        sl = slice(512 * j, 512 * (j + 1))
        if j == 0:
            xs = xs0
        else:
            xs = []
            for k in range(NK):
                xt = xpool.tile([128, 512], BF16, name=f"x_{j}_{k}", tag="xs")
                eng = dma_a if k % 2 == 0 else dma_b
                eng.dma_start(out=xt, in_=xT[128 * k:128 * (k + 1), sl])
                xs.append(xt)
        for w_sb, dsts, tagn in ((wq_sb, qT, "q"), (wk_sb, kT, "k")):
            accs = [
                pspool.tile(
                    [128, 512], F32, name=f"acc{tagn}_{j}_{h}", tag="acc"
                )
                for h in range(HPC)
            ]
            for k in range(NK):
                for h in range(HPC):
                    nc.tensor.matmul(
                        accs[h], w_sb[k][:, 128 * h:128 * (h + 1)], xs[k],
                        start=(k == 0), stop=(k == NK - 1),
                    )
            for h in range(HPC):
                acc, dst = accs[h], dsts[h]
                shuf = tpool.tile([128, 512], F32, name=f"sh{tagn}_{j}_{h}", tag="shuf")
                nc.vector.stream_shuffle(shuf, acc, SWAP_MASK)
                # GpSimd cannot read PSUM: it takes the SBUF-side mul
                # (shuf*sin); DVE reads acc straight from PSUM for cos.
                t1 = tpool.tile([128, 512], BF16, name=f"t1{tagn}_{j}_{h}", tag="t1")
                nc.gpsimd.tensor_mul(t1, shuf, sin_sb[:, sl])
                t2 = tpool.tile([128, 512], BF16, name=f"t2{tagn}_{j}_{h}", tag="t2")
                nc.vector.tensor_mul(t2, acc, cos_sb[:, sl])
                nc.vector.tensor_add(dst[:, sl], t1, t2)
    wq_pool.release()

    # ---- phase 1b: v projection in natural [seq, head_dim] layout
    # wv tiles recycle the wk pool's slots; accv tiles recycle the ph1a psum
    # tag — both avoid pool-boundary serialization at the phase seam.
    wv_sb = []
    for k in range(NK):
        wt = wk_pool.tile([128, HPC * HD], BF16, name=f"wv{k}", tag="wk")
        dma_b.dma_start(out=wt, in_=wvT[128 * k:128 * (k + 1), :])
        wv_sb.append(wt)

    v_pool = tc.alloc_tile_pool(name="vp", bufs=NSK, side="right")
    v_sb = [v_pool.tile([128, HPC * HD], BF16, name=f"v{m}", tag="v") for m in range(NSK)]
    for m in range(NSK):
        xcp = []
        for g in range(4):
            xt = xpool.tile([128, 4, 128], BF16, name=f"xc{m}_{g}", tag="xs")
            eng = dma_a if g % 2 == 0 else dma_b
            eng.dma_start(
                out=xt,
                in_=xV[128 * m:128 * (m + 1), 512 * g:512 * (g + 1)].rearrange(
                    "p (kt c) -> p kt c", c=128
                ),
            )
            xcp.append(xt)
        acc = pspool.tile([128, HPC * HD], F32, name=f"accv{m}", tag="acc")
        for k in range(NK):
            nc.tensor.matmul(
                acc, xcp[k // 4][:, k % 4, :], wv_sb[k],
                start=(k == 0), stop=(k == NK - 1),
            )
        nc.vector.tensor_copy(out=v_sb[m], in_=acc)
    xpool.release()
    pspool.release()
    wk_pool.release()
    tpool.release()
    ropec.release()

    # ---- phase 2: causal attention per head, transposed-score layout
    oh_pool = tc.alloc_tile_pool(name="ohp", bufs=HPC, side="right")
    out_hT = [oh_pool.tile([128, S], BF16, name=f"oh{h}", tag="oh") for h in range(HPC)]
    # prefetch wo during attention
    wo_pool = tc.alloc_tile_pool(name="wop", bufs=HPC, side="right")
    wo_sb = []
    for h in range(HPC):
        wt = wo_pool.tile([128, D], BF16, name=f"wo{h}", tag="wo")
        dma_a.dma_start(out=wt, in_=woT[128 * h:128 * (h + 1), :])
        wo_sb.append(wt)

    pp = tc.alloc_tile_pool(name="pp", bufs=8)
    small2 = tc.alloc_tile_pool(name="small2", bufs=4)
    pss = tc.alloc_tile_pool(name="pss", bufs=5, space="PSUM")
    psd = tc.alloc_tile_pool(name="psd", bufs=1, space="PSUM")
    pspv = tc.alloc_tile_pool(name="pspv", bufs=2, space="PSUM")
    for h in range(HPC):
        for j in range(NSQ):
            sl = slice(512 * j, 512 * (j + 1))
            nsk = 4 * j + 4
            den = psd.tile([128, 512], F32, name=f"den{h}_{j}", tag="den")
            pv = pspv.tile([128, 512], F32, name=f"pv{h}_{j}", tag="pv")
            # diagonal (masked) tiles first: their exp->mask latency hides
            # under the unmasked tiles' pv matmuls.  The denominator is the
            # elementwise sum of all pt tiles on TWO chains (GpSimd takes
            # even positions, DVE odd), merged by the final DVE add into a
            # bf16 tile, reduced over partitions by ONE ones-matmul.
            order = list(range(4 * j, nsk)) + list(range(0, 4 * j))
            # diagonal tile with mask pattern m: columns sql < 128*m are
            # fully masked — compute only a column slice
            offs = {0: 0, 1: 128, 2: 256, 3: 256}
            pacc_g = None
            pacc_v = None
            pts = []
            for idx, i in enumerate(order):
                off = offs[i - 4 * j] if i >= 4 * j else 0
                cs = slice(off, 512)
                qs = slice(512 * j + off, 512 * (j + 1))
                s_ps = pss.tile([128, 512], F32, name=f"s{h}_{j}_{i}", tag="s")
                nc.tensor.matmul(
                    s_ps[:, cs], kT[h][:, 128 * i:128 * (i + 1)], qT[h][:, qs],
                    start=True, stop=True,
                )
                pt = pp.tile([128, 512], BF16, name=f"p{h}_{j}_{i}", tag="pt")
                nc.scalar.activation(pt[:, cs], s_ps[:, cs], EXP, bias=0.0, scale=SCALE)
                if i >= 4 * j:
                    nc.vector.tensor_mul(pt[:, cs], pt[:, cs], mask_sb[i - 4 * j][:, cs])
                nc.tensor.matmul(
                    pv[:, cs], v_sb[i][:, 128 * h:128 * (h + 1)], pt[:, cs],
                    start=(idx == 0), stop=(idx == nsk - 1),
                )
                pts.append((pt, off))
                # denominator: two parallel accumulation chains
                if idx == nsk - 1:
                    nc.vector.tensor_add(pacc_v[:, cs], pacc_v[:, cs], pt[:, cs])
                    pacc_r = small2.tile(
                        [128, 512], BF16, name=f"par{h}_{j}", tag="paccr", bufs=2
                    )
                    nc.vector.tensor_add(pacc_r, pacc_g, pacc_v)
                    nc.tensor.matmul(den, ones_sq, pacc_r, start=True, stop=True)
                elif idx == 0:
                    pacc_g = small2.tile(
                        [128, 512], F32, name=f"pag{h}_{j}", tag="pacc_g", bufs=2
                    )
                    if off > 0:
                        nc.gpsimd.memset(pacc_g[:, 0:off], 0.0)
                    nc.gpsimd.tensor_copy(out=pacc_g[:, cs], in_=pt[:, cs])
                elif idx == 1:
                    pacc_v = small2.tile(
                        [128, 512], F32, name=f"pav{h}_{j}", tag="pacc_v", bufs=2
                    )
                    if off > 0:
                        nc.vector.memset(pacc_v[:, 0:off], 0.0)
                    nc.vector.tensor_copy(out=pacc_v[:, cs], in_=pt[:, cs])
                elif idx % 2 == 0:
                    nc.gpsimd.tensor_add(pacc_g[:, cs], pacc_g[:, cs], pt[:, cs])
                else:
                    nc.vector.tensor_add(pacc_v[:, cs], pacc_v[:, cs], pt[:, cs])
            recip = small2.tile([128, 512], F32, name=f"rc{h}_{j}", tag="recip")
            scr = small2.tile([128, 512], F32, name=f"sc{h}_{j}", tag="scr", bufs=2)
            nc.vector.reciprocal_approx_accurate(recip, den, scr)
            nc.vector.tensor_mul(out_hT[h][:, sl], pv, recip)
    pspv.release()
    psd.release()
    pss.release()
    small2.release()
    pp.release()
    qk_pool.release()

    # ---- phase 3: row-parallel wo partial product, row-block output DMAs
    ys_pool = tc.alloc_tile_pool(name="ysp", bufs=3)
    psy_pool = tc.alloc_tile_pool(name="psy", bufs=3, space="PSUM")
    for t in range(NSK):
        ys = ys_pool.tile([128, D], F32, name=f"ys{t}", tag="ys")
        for n in range(NSQ):
            acc = psy_pool.tile([128, 512], F32, name=f"accy{t}_{n}", tag="y")
            for h in range(HPC):
                nc.tensor.matmul(
                    acc,
                    out_hT[h][:, 128 * t:128 * (t + 1)],
                    wo_sb[h][:, 512 * n:512 * (n + 1)],
                    start=(h == 0),
                    stop=(h == HPC - 1),
                )
            nc.vector.tensor_copy(out=ys[:, 512 * n:512 * (n + 1)], in_=acc)
        dma_a.dma_start(out=y[128 * t:128 * (t + 1), :], in_=ys)
    psy_pool.release()
    ys_pool.release()
    wo_pool.release()
    oh_pool.release()
    v_pool.release()
    consts.release()


_PROGRAM = None


def build_program():
    global _PROGRAM
    if _PROGRAM is None:
        nc = bacc.Bacc("TRN2", target_bir_lowering=False, debug=False)
        with tile.TileContext(nc) as tc:
            _emit(tc)
        nc.compile()
        _PROGRAM = nc
    return _PROGRAM


def make_core_inputs(x, freqs_cos, freqs_sin, wq, wk, wv, wo):
    """Host-side sharding: returns list of 8 per-core input dicts."""
    x = np.asarray(x, dtype=np.float32)
    freqs_cos = np.asarray(freqs_cos, dtype=np.float32)
    freqs_sin = np.asarray(freqs_sin, dtype=np.float32)
    wq = np.asarray(wq, dtype=np.float32)
    wk = np.asarray(wk, dtype=np.float32)
    wv = np.asarray(wv, dtype=np.float32)
    wo = np.asarray(wo, dtype=np.float32)

    cosq = np.ascontiguousarray(np.repeat(freqs_cos.T, 2, axis=0))  # [128, S]
    sinq = np.ascontiguousarray(np.repeat(freqs_sin.T, 2, axis=0))
    sinq[0::2, :] *= -1.0  # even rows: -sin; odd rows: +sin

    skl = np.arange(128)[:, None]
    sql = np.arange(512)[None, :]
    dmask = np.stack(
        [(128 * m + skl <= sql).astype(NPBF) for m in range(4)]
    )  # [4, 128, 512]

    onesd = np.ones((128, 128), dtype=NPBF)
    xTs = [np.ascontiguousarray(x[b].T).astype(NPBF) for b in range(B)]
    # V-phase layout: xV[128m+p, 128kt+c] = x[b][128m+c, 128kt+p]
    xVs = [
        np.ascontiguousarray(
            np.asarray(xr, np.float32)
            .T.reshape(16, 128, 16, 128)
            .transpose(0, 3, 2, 1)
            .reshape(2048, 2048)
        ).astype(NPBF)
        for xr in xTs
    ]
    in_maps = []
    for c in range(N_CORES):
        b, g = divmod(c, CPB)
        hsl = slice(512 * g, 512 * (g + 1))
        in_maps.append(
            {
                "xT": xTs[b],
                "xV": xVs[b],
                "wqT": np.ascontiguousarray(wq[hsl, :].T).astype(NPBF),
                "wkT": np.ascontiguousarray(wk[hsl, :].T).astype(NPBF),
                "wvT": np.ascontiguousarray(wv[hsl, :].T).astype(NPBF),
                "woT": np.ascontiguousarray(wo[:, hsl].T).astype(NPBF),
                "cosq": cosq,
                "sinq": sinq,
                "dmask": dmask,
                "onesd": onesd,
            }
        )
    return in_maps


def run(inputs, trace=False, **spmd_kwargs):
    """Run the SPMD kernel on 8 cores.  Returns (y_full, BassKernelResults)."""
    nc = build_program()
    in_maps = make_core_inputs(
        inputs["x"], inputs["freqs_cos"], inputs["freqs_sin"],
        inputs["wq"], inputs["wk"], inputs["wv"], inputs["wo"],
    )
    res = bass_utils.run_bass_kernel_spmd(
        nc, in_maps, list(range(N_CORES)), trace=trace, **spmd_kwargs
    )
    out = np.zeros((B, S, D), dtype=np.float32)
    for c in range(N_CORES):
        out[c // CPB] += res.results[c]["y"]
    return out, res


def kernel(**inputs):
    out, _ = run(inputs, trace=False)
    return out


def simulate_core(core_idx, inputs):
    """CoreSim-validate a single core's program; returns its partial y."""
    from concourse.bass_interp import CoreSim

    nc = build_program()
    in_maps = make_core_inputs(
        inputs["x"], inputs["freqs_cos"], inputs["freqs_sin"],
        inputs["wq"], inputs["wk"], inputs["wv"], inputs["wo"],
    )
    sim = CoreSim(nc)
    for name, arr in in_maps[core_idx].items():
        sim.tensor(name)[:] = arr
    sim.simulate()
    return np.array(sim.tensor("y"))
